# revision 30
# baseline (speedup 1.0000x reference)
"""CDSSM kernel for Trainium2 (8 NeuronCores, data-parallel over batch).

Model (per sequence of T=2048, D=128):
  h1 = tanh(conv1d(x^T, w1, b1))        # [K=128,  T-2]  (FL=3, VALID)
  h2 = tanh(conv1d(h1, w2, b2))         # [K2=128, T-4]
  hmax = max_t(h2)                      # k-max pooling, k=1
  s = tanh(sw @ hmax + sb)              # [L=64]
Then dots q.s/d.s, scale by gamma, softmax over [pos, n0, n1].

Sharding: B=64 split 8 ways; each core encodes 32 sequences
(8 q with q-weights, 8 pos + 16 negs with d-weights).

Convs run in fp8e4 (e4m3) with DoubleRow perf mode: activations are
parity-split on host (x -> [even cols | odd cols] slabs) so each pair of
conv taps maps onto the two DoubleRow halves with HW-legal (aligned)
AP strides.  3 taps = 2 DoubleRow matmuls per output block (one half of
the second pair is zero weights).  This runs the PE at 0.5 cycles/col.
Weights are pre-scaled by 32 to dodge fp8 subnormals; the inverse scale
is folded into the ACT tanh (conv1) / final tanh (conv2).

Per-seq engine budget: PE ~1.7us (convs), ACT ~2.1us (2 tanh megas),
DVE ~2.4us (2 PSUM max-reduces) -> DVE-bound pipeline.
PSUM: conv1 2x[128,2,512] double-buffered (4 banks) + conv2 same
(4 banks) = 8 banks; final linear/softmax matmuls write into corners of
the last conv2 megas (no 9th bank).
"""

import numpy as np

B, T, D = 64, 2048, 128
K, K2, L, FL, J = 128, 128, 64, 3, 2
NCORES = 8
BPC = B // NCORES          # sequences of each role per core
NSEQ = 4 * BPC             # 32 slabs per core: [q x8 | pos x8 | n0 x8 | n1 x8]
U = T // 2                 # 1024 parity columns
WS = 32.0                  # fp8 weight pre-scale
NPAIR = 4                  # pair slabs per (conv, e): PA, PB, PC, PD
SLAB_F32 = 64              # f32 cols per [128,2,128] fp8 pair slab
NE = 2 * NPAIR * SLAB_F32  # 512 f32 cols of conv slabs per weight set (q/d)
# blob layout: [q slabs 512 | biases 6 | swt 128 | d slabs 512]; the first
# 518 cols ship in a small fast DMA so the PE can start early
NB1 = NE + 6
NBLOB = NB1 + 128 + NE


def _slab_col(conv, e, p):
    return (NB1 + 128) * e + (conv * NPAIR + p) * SLAB_F32

# module-level handle for test harness introspection (exec time / profile)
LAST_RESULTS = None


def _build_program(gw_val: float, reps: int = 1):
    import concourse.bacc as bacc
    import concourse.tile as tile
    from concourse import mybir

    f32 = mybir.dt.float32
    f8 = mybir.dt.float8e4
    bf16 = mybir.dt.bfloat16
    AFT = mybir.ActivationFunctionType
    AX = mybir.AxisListType
    ALU = mybir.AluOpType
    DR = mybir.MatmulPerfMode.DoubleRow

    nc = bacc.Bacc()

    def _observe(eng, ins=(), outs=()):
        # NoOp with APs: Tile wires the deps onto it, so the wait lands here
        # instead of on the next (wait-slot-limited) instruction.
        inst = mybir.InstNoOp(
            name=nc.get_next_instruction_name(), text_hint="obs",
            bass_nofuse=True,
        )
        inst.ins = [eng.lower_ap(ap) for ap in ins]
        inst.outs = [eng.lower_ap(ap) for ap in outs]
        return eng.add_instruction(inst)

    x_d = nc.dram_tensor("x", [NSEQ, 2, D, U], f8, kind="ExternalInput")
    blob_d = nc.dram_tensor("blob", [128, NBLOB], f32, kind="ExternalInput")
    out_d = nc.dram_tensor("out", [1, 3 * BPC], f32, kind="ExternalOutput")

    with tile.TileContext(nc) as tc:
        with (
            tc.tile_pool(name="consts", bufs=1) as consts,
            tc.tile_pool(name="xp", bufs=1) as xp,
            tc.tile_pool(name="h1p", bufs=1) as h1p,
            tc.tile_pool(name="smallp", bufs=1) as smallp,
            tc.tile_pool(name="ps1", bufs=2, space="PSUM") as ps1,
            tc.tile_pool(name="ps2", bufs=2, space="PSUM") as ps2,
        ):
            # --- weights / constants in SBUF (2 DMAs: fast head, slow rest;
            # the tail is emitted later so x chunks 1-3 precede it in the
            # serial DMA stream) ---
            blob = consts.tile([128, NBLOB], f32)
            b1t = blob[:, NE:NE + 2]
            b2t = blob[:, NE + 2:NE + 4]
            sbt = blob[0:L, NE + 4:NE + 6]
            swt = blob[:, NB1:NB1 + 128]      # cols 0:64 q, 64:128 d (fp32)

            # conv pair slabs: [conv][e][pair] -> [128, 2, 128] fp8
            wp = [[[None] * NPAIR for _ in range(2)] for _ in range(2)]
            for conv in range(2):
                for e in range(2):
                    for p in range(NPAIR):
                        c0 = _slab_col(conv, e, p)
                        wp[conv][e][p] = blob[:, c0:c0 + SLAB_F32].bitcast(
                            f8).rearrange("d (two k) -> d two k", k=128)

            ones = consts.tile([L, 1], f32)
            nc.vector.memset(ones, 1.0)

            # warm-up matmuls (absorb the blob-DMA queue wait on PE); write
            # into a throwaway corner of the first conv1 PSUM tile -- the
            # first real matmul's start=True resets it.
            megaW = ps1.tile([128, 2, 512], f32, name="m1")
            w0ap = wp[0][0][0]
            nc.tensor.matmul(megaW[0:1, 0, 0:2], w0ap[:, 0, 0:1],
                             w0ap[:, 0, 0:2], start=True, stop=True,
                             skip_group_check=True)
            nc.tensor.matmul(megaW[0:1, 0, 0:2], w0ap[:, 1, 0:1],
                             w0ap[:, 1, 0:2], start=True, stop=True,
                             skip_group_check=True)

            # ACT table warm-ups + bias-DMA observers (keep real activations
            # at <=1 sync wait)
            trash = consts.tile([128, 8], f32)
            nc.vector.memset(trash, 0.0)
            nc.scalar.activation(trash[:, 0:1], trash[:, 1:2], AFT.Tanh)
            nc.scalar.activation(trash[:, 2:3], trash[:, 3:4], AFT.Exp)
            nc.scalar.copy(trash[:, 4:5], b1t[:, 0:1])
            nc.scalar.copy(trash[:, 5:6], b2t[:, 0:1])
            nc.scalar.copy(trash[0:L, 6:7], sbt[:, 0:1])

            Ha = smallp.tile([128, NSEQ], f32)
            Hb = smallp.tile([128, NSEQ], f32)
            # persistent double-buffered h1 ring (fp8, parity layout)
            h1full = h1p.tile([128, 2, 2, U], f8)
            # all 32 activation slabs resident (fp8, parity layout)
            xfull = xp.tile([128, NSEQ, 2, U], f8)

            # x loads in chunks sized to stay ahead of the ~2.4us/seq compute
            # on the serial DMA stream; tiny first chunk so the PE starts
            # almost immediately.  The slow blob half slots in after chunk 3.
            chunks = [0, 1, 2, 4, 8, 17, NSEQ]

            def chunk_dma(ci):
                a, b = chunks[ci], chunks[ci + 1]
                nc.sync.dma_start(
                    out=xfull[:, a:b],
                    in_=x_d[a:b].rearrange("s p d u -> d s p u"))

            # first slab in two halves: ma(0) only needs u cols [0:513]
            nc.sync.dma_start(out=xfull[:, 0:1, :, 0:520],
                              in_=x_d[0:1, :, :, 0:520].rearrange(
                                  "s p d u -> d s p u"))
            nc.sync.dma_start(out=blob[:, 0:NE // 2],
                              in_=blob_d[:, 0:NE // 2])
            nc.sync.dma_start(out=xfull[:, 0:1, :, 520:U],
                              in_=x_d[0:1, :, :, 520:U].rearrange(
                                  "s p d u -> d s p u"))
            nc.sync.dma_start(out=blob[:, NE // 2:NB1],
                              in_=blob_d[:, NE // 2:NB1])
            for ci in range(1, 3):
                chunk_dma(ci)
            nc.sync.dma_start(out=blob[:, NB1:], in_=blob_d[:, NB1:])
            for ci in range(3, 6):
                chunk_dma(ci)

            def conv_block(mega, half, w4, src, u0, n, dr_shift):
                """One parity block: 2 DoubleRow matmuls into mega[:,half,0:n].
                w4 = (first_pair, second_pair); dr_shift = rhs offsets."""
                o = mega[:, half, 0:n]
                nc.tensor.matmul(o, w4[0], src[:, :, u0 + dr_shift[0]:
                                               u0 + dr_shift[0] + n],
                                 start=True, stop=False, perf_mode=DR)
                nc.tensor.matmul(o, w4[1], src[:, :, u0 + dr_shift[1]:
                                               u0 + dr_shift[1] + n],
                                 start=False, stop=True, perf_mode=DR)

            def conv_mega(pool, wps, src, u0, n, name):
                """One PSUM mega [even-block | odd-block] at u offset u0."""
                mega = pool.tile([128, 2, 512], f32, name=name)
                PA, PB, PC, PD = wps
                conv_block(mega, 0, (PA, PB), src, u0, n, (0, 1))
                conv_block(mega, 1, (PC, PD), src, u0, n, (1, 0))
                return mega

            def emit_conv2(s):
                e = 0 if s < BPC else 1
                h1 = h1full[:, s % 2]
                # per-mega carriers absorb the h1-ready (ACT) waits so conv2
                # matmuls only wait on their PSUM slot release (DVE)
                _observe(nc.tensor, ins=[h1[:, 0, 0:1]])
                mc = conv_mega(ps2, wp[1][e], h1, 0, 511, "m2")
                nc.vector.tensor_reduce(Ha[:, s:s + 1], mc[:, 0:2, 0:511],
                                        axis=AX.XY, op=ALU.max)
                _observe(nc.tensor, ins=[h1[:, 0, 512:513]])
                md = conv_mega(ps2, wp[1][e], h1, 511, 511, "m2")
                nc.vector.tensor_reduce(Hb[:, s:s + 1], md[:, 0:2, 0:511],
                                        axis=AX.XY, op=ALU.max)
                return mc, md

            for s in range(NSEQ):
                e = 0 if s < BPC else 1
                xs = xfull[:, s]
                if s in chunks:
                    # absorb the x-chunk DMA wait
                    _observe(nc.tensor, ins=[xs[:, 0, 0:1]])
                if s == BPC:
                    # absorb the d-weights (slow blob half) DMA wait
                    _observe(nc.tensor, ins=[wp[0][1][0][:, 0, 0:1]])
                ma = conv_mega(ps1, wp[0][e], xs, 0, 512, "m1")
                if s == 0:
                    # second half-slab DMA wait, absorbed before mb(0)
                    _observe(nc.tensor, ins=[xs[:, 0, U - 1:U]])
                mb = conv_mega(ps1, wp[0][e], xs, 512, 511, "m1")
                h1 = h1full[:, s % 2]
                nc.scalar.activation(h1[:, 0:2, 0:512], ma[:, 0:2, 0:512],
                                     AFT.Tanh, bias=b1t[:, e:e + 1],
                                     scale=1.0 / WS)
                nc.scalar.activation(h1[:, 0:2, 512:1023], mb[:, 0:2, 0:511],
                                     AFT.Tanh, bias=b1t[:, e:e + 1],
                                     scale=1.0 / WS)
                # conv2 emission: seq 0 immediately (early DVE start), then
                # a 1-seq pipeline shift so conv2(s-1) sits behind conv1(s)
                # in the PE queue with its ACT deps already drained
                if s == 0:
                    emit_conv2(0)
                elif s >= 2:
                    emit_conv2(s - 1)
            mc31, md31 = emit_conv2(NSEQ - 1)

            # --- final linear + dots + softmax (tiny, exact fp32) ---
            nq = BPC
            Hm = smallp.tile([128, NSEQ], f32)
            nc.vector.tensor_tensor(Hm, Ha, Hb, op=ALU.max)
            # Htan = tanh(Hm/WS + b2); max commutes with the monotone tanh
            Htan = smallp.tile([128, NSEQ], f32)
            nc.scalar.activation(Htan[:, 0:nq], Hm[:, 0:nq], AFT.Tanh,
                                 bias=b2t[:, 0:1], scale=1.0 / WS)
            nc.scalar.activation(Htan[:, nq:NSEQ], Hm[:, nq:NSEQ], AFT.Tanh,
                                 bias=b2t[:, 1:2], scale=1.0 / WS)
            # final linear into corners of the (drained) conv2 megas
            sps = mc31[0:L, 0, 0:NSEQ]
            nc.tensor.matmul(sps[:, 0:nq], swt[:, 0:L], Htan[:, 0:nq],
                             start=True, stop=True)
            nc.tensor.matmul(sps[:, nq:NSEQ], swt[:, L:128], Htan[:, nq:NSEQ],
                             start=True, stop=True)
            S = smallp.tile([L, NSEQ], f32)
            nc.scalar.activation(S[:, 0:nq], sps[:, 0:nq], AFT.Tanh,
                                 bias=sbt[:, 0:1])
            nc.scalar.activation(S[:, nq:NSEQ], sps[:, nq:NSEQ], AFT.Tanh,
                                 bias=sbt[:, 1:2])

            M = smallp.tile([L, 3 * nq], f32)
            for j in range(3):
                nc.vector.tensor_mul(M[:, j * nq:(j + 1) * nq],
                                     S[:, 0:nq],
                                     S[:, (j + 1) * nq:(j + 2) * nq])
            dps = md31[0:1, 0, 0:3 * nq]
            nc.tensor.matmul(dps, ones, M, start=True, stop=True)

            # E = exp(gw * dots); gb cancels in softmax and the
            # max-subtraction is unnecessary (|dots| <= 64 fits exp fp32)
            E = smallp.tile([1, 3 * nq], f32)
            nc.scalar.activation(E, dps, AFT.Exp, scale=float(gw_val))
            ssum = smallp.tile([1, nq], f32)
            nc.vector.tensor_add(ssum, E[:, 0:nq], E[:, nq:2 * nq])
            nc.vector.tensor_add(ssum, ssum, E[:, 2 * nq:3 * nq])
            rec = smallp.tile([1, nq], f32)
            nc.vector.reciprocal(rec, ssum)
            O = smallp.tile([1, 3 * nq], f32)
            for j in range(3):
                nc.vector.tensor_mul(O[:, j * nq:(j + 1) * nq],
                                     E[:, j * nq:(j + 1) * nq], rec)
            nc.sync.dma_start(out=out_d[:, :], in_=O)

    nc.compile()
    return nc


def _host_prep(q, pos, negs, qw1, qb1, qw2, qb2, qsw, qsb,
               dw1, db1, dw2, db2, dsw, dsb, gw, gb):
    import ml_dtypes
    f = np.float32
    f8 = ml_dtypes.float8_e4m3

    # per-core activation slabs: [core, seq, parity, d, u] in e4m3
    def slabs(x):
        # x: [NCORES*BPC, T, D] fp32 -> [NCORES, BPC, 2, D, U]
        x8 = np.asarray(x, f).astype(f8)
        x8 = x8.reshape(NCORES, BPC, U, 2, D)
        return x8.transpose(0, 1, 3, 4, 2)     # [c, s, p, d, u]

    xall = np.empty((NCORES, NSEQ, 2, D, U), f8)
    xall[:, 0:BPC] = slabs(np.asarray(q, f))
    xall[:, BPC:2 * BPC] = slabs(np.asarray(pos, f))
    xall[:, 2 * BPC:3 * BPC] = slabs(np.asarray(negs[0], f))
    xall[:, 3 * BPC:4 * BPC] = slabs(np.asarray(negs[1], f))

    # conv weight pair slabs, pre-scaled by WS, e4m3:
    # PA=(W0,W1), PB=(W2,0), PC=(W1,W2), PD=(0,W0), W_f = w[:, :, f].T
    def pair_slabs(w):
        Wf = [(np.asarray(w, f)[:, :, fi].T * WS).astype(f8) for fi in range(FL)]
        Z = np.zeros_like(Wf[0])
        return [np.stack(p, axis=1) for p in
                ((Wf[0], Wf[1]), (Wf[2], Z), (Wf[1], Wf[2]), (Z, Wf[0]))]

    blob = np.zeros((128, NBLOB), f)
    blob[:, NE] = np.asarray(qb1, f)
    blob[:, NE + 1] = np.asarray(db1, f)
    blob[:, NE + 2] = np.asarray(qb2, f)
    blob[:, NE + 3] = np.asarray(db2, f)
    blob[0:L, NE + 4] = np.asarray(qsb, f)
    blob[0:L, NE + 5] = np.asarray(dsb, f)
    blob[:, NB1:NB1 + 64] = np.asarray(qsw, f).T
    blob[:, NB1 + 64:NB1 + 128] = np.asarray(dsw, f).T
    for conv, (wq, wd) in enumerate(((qw1, dw1), (qw2, dw2))):
        for e, w in enumerate((wq, wd)):
            for p, slab in enumerate(pair_slabs(w)):
                c0 = _slab_col(conv, e, p)
                blob[:, c0:c0 + SLAB_F32] = np.ascontiguousarray(
                    slab.reshape(128, 256)).view(np.uint8).view(f)

    in_maps = [{"blob": blob, "x": xall[c]} for c in range(NCORES)]
    return in_maps, float(np.asarray(gw, f))


def _assemble(results):
    final = np.empty((B, 3), np.float32)
    for c in range(NCORES):
        o = results[c]["out"][0]              # [3*BPC], j-major
        final[c * BPC:(c + 1) * BPC, :] = o.reshape(3, BPC).T
    return final


def kernel(**inputs):
    global LAST_RESULTS
    from concourse import bass_utils

    in_maps, gw_val = _host_prep(**inputs)
    nc = _build_program(gw_val)
    res = bass_utils.run_bass_kernel_spmd(nc, in_maps, core_ids=list(range(NCORES)))
    LAST_RESULTS = res
    return _assemble(res.results)
